# revision 27
# baseline (speedup 1.0000x reference)
"""Trainium2 Bass kernel for the ReActNet-style binary conv building block.

Data-parallel across 8 NeuronCores (8 samples each). Key structure per
2-sample group:
  - scalar Sign act binarizes x -> fp8 +/-1 planes (zero-padded ring)
  - conv1 runs as 9 fp8 DoubleRow matmuls per half (K=256 per instr)
  - the 2x2 avgpool shortcut: bf16 quant grid (bf16(7.5x+199.5) rounds
    exactly), clipped, then summed by diag(E1) matmuls into PSUM
  - BN+RPReLU+shortcut-BN fold into a single per-channel Prelu
    activation (alpha = beta vector) per conv tile
  - stage 2: 1x1 conv as one fp8 DoubleRow matmul per 128-channel tile
  - final combine on DVE in bf16; output stored bf16, host casts to f32
"""

import sys

sys.path.insert(0, "/opt/trn_rl_repo")

import numpy as np
import ml_dtypes

B_PER_CORE = 8
N_CORES = 8
CIN = 256
COUT = 512
H = 28
W = 28
HO = 14
WO = 14
PIX = HO * WO  # 196
NG = 2  # samples per group
NGROUP = 4  # groups per core
NCOL = NG * PIX  # 392

# padded image layout rows 0..29, cols 0..31; interior at [1:29, 2:30]
PH, PW = 30, 32

_PROGRAM_CACHE = {}


def _build_program():
    if "nc" in _PROGRAM_CACHE:
        return _PROGRAM_CACHE["nc"]

    import concourse.bacc as bacc
    import concourse.tile as tile
    from concourse import mybir

    f32 = mybir.dt.float32
    bf16 = mybir.dt.bfloat16
    fp8 = mybir.dt.float8e4
    Alu = mybir.AluOpType
    Act = mybir.ActivationFunctionType
    DR = mybir.MatmulPerfMode.DoubleRow

    nc = bacc.Bacc(
        "TRN2",
        target_bir_lowering=False,
        debug=False,
        enable_asserts=False,
        num_devices=N_CORES,
    )

    xs_d = nc.dram_tensor("xs", [B_PER_CORE, 2, 128, H * W], f32, kind="ExternalInput")
    w3_d = nc.dram_tensor("w3f", [128, 9, 2, 2, 128], fp8, kind="ExternalInput")
    w1_d = nc.dram_tensor("w1f", [128, 2, 4, 128], fp8, kind="ExternalInput")
    dg_d = nc.dram_tensor("dg", [128, 128], bf16, kind="ExternalInput")
    cv_d = nc.dram_tensor("cv", [128, 33], f32, kind="ExternalInput")
    out_d = nc.dram_tensor(
        "out", [4, 128, B_PER_CORE, PIX], bf16, kind="ExternalOutput"
    )

    with tile.TileContext(nc) as tc:
        with (
            tc.tile_pool(name="consts", bufs=1) as cpool,
            tc.tile_pool(name="xin", bufs=2) as xpool,
            tc.tile_pool(name="rq", bufs=3) as rpool,
            tc.tile_pool(name="bpad", bufs=3) as bpool,
            tc.tile_pool(name="p1s", bufs=3) as p1pool,
            tc.tile_pool(name="ys", bufs=3) as ypool,
            tc.tile_pool(name="q2s", bufs=3) as qpool,
            tc.tile_pool(name="s2s", bufs=3) as spool,
            tc.tile_pool(name="p2s", bufs=3) as p2pool,
            tc.tile_pool(name="zs", bufs=3) as zpool,
            tc.tile_pool(name="pc1", bufs=2, space="PSUM") as pc1,
            tc.tile_pool(name="pq", bufs=2, space="PSUM") as pq,
            tc.tile_pool(name="pc2", bufs=2, space="PSUM") as pc2,
        ):
            W3F = cpool.tile([128, 9, 2, 2, 128], fp8)
            W1F = cpool.tile([128, 2, 4, 128], fp8)
            DG = cpool.tile([128, 128], bf16)
            CV = cpool.tile([128, 33], f32)
            nc.sync.dma_start(W3F[:], w3_d[:])
            nc.sync.dma_start(W1F[:], w1_d[:])
            nc.sync.dma_start(DG[:], dg_d[:])
            nc.sync.dma_start(CV[:], cv_d[:])

            def cvec(col):
                return CV[:, col : col + 1]

            # cv columns: 0,1 sA1 | 2,3 bA1 | 4,5 beta1 | 6,7 D1 | 8,9 r2bias
            # 10-13 sA2 | 14-17 bA2 | 18-21 beta2 | 22-25 E2 | 26-29 D2
            # 30,31 E1 | 32 -D1 (j=1, is_ge threshold)

            for g in range(NGROUP):
                X = xpool.tile([128, 2, NG, H * W], f32, tag="x")
                for si in range(NG):
                    nc.sync.dma_start(
                        X[:, :, si, :],
                        xs_d[NG * g + si].rearrange("c p hw -> p c hw"),
                    )
                Xv = X[:].rearrange("p c s (h w) -> p c s h w", h=H, w=W)

                BP = bpool.tile([128, 2, NG, PH, PW], fp8, tag="bpad")
                if g < 3:  # one memset per rotating buffer (bufs=3)
                    # ring cells the conv taps read; interior rewritten
                    # each group, ring stays zero across buffer reuse
                    nc.gpsimd.memset(BP[:, :, :, 0, :], 0.0)
                    nc.gpsimd.memset(BP[:, :, :, 1:29, 1], 0.0)
                # binarize: sign(x) in {-1,+1} -> fp8, zero-padded ring.
                # Per-sample grains so compute starts after half an X load.
                R = rpool.tile([128, 2, NG, H * W], bf16, tag="r")
                RC = rpool.tile([128, 2, NG, H, W], bf16, tag="rc")
                for si in range(NG):
                    nc.scalar.activation(
                        BP[:, :, si, 1:29, 2:30], Xv[:, :, si], Act.Sign
                    )
                    # quant grid: bf16(7.5x+199.5) rounds to ints; clip
                    nc.vector.tensor_scalar(
                        R[:, :, si, :], X[:, :, si, :], 7.5, 199.5,
                        Alu.mult, Alu.add,
                    )
                    nc.vector.tensor_scalar(
                        RC[:, :, si, :, :],
                        R[:, :, si, :].rearrange("p c (h w) -> p c h w", h=H),
                        207.0, 192.0, Alu.min, Alu.max,
                    )

                # 2x2 sum-pool via identity matmuls into PSUM (exact ints)
                Q2p = [
                    pq.tile([128, 512], f32, tag=f"pq{j}", name=f"q2p_{g}_{j}")
                    for j in range(2)
                ]
                for j in range(2):
                    om = Q2p[j][:, :NCOL].rearrange(
                        "p (s y x) -> p s y x", s=NG, y=HO
                    )
                    for pp in range(4):
                        ph, pw = pp >> 1, pp & 1
                        nc.tensor.matmul(
                            om,
                            DG[:],
                            RC[:, j, :, ph::2, pw::2],
                            start=(pp == 0),
                            stop=(pp == 3),
                        )

                # conv1: 9 fp8 DoubleRow matmuls per half (K=256 each)
                y4 = ypool.tile([128, 2, NCOL], f32, tag="y4")
                S24 = spool.tile([128, 2, NCOL], fp8, tag="s24")
                for j in range(2):
                    ps1 = pc1.tile([128, 512], f32, tag="ps1")
                    for si in range(NG):
                        om = ps1[:, si * PIX : (si + 1) * PIX].rearrange(
                            "p (y x) -> p y x", y=HO
                        )
                        for t in range(9):
                            kh, kw = t // 3, t % 3
                            nc.tensor.matmul(
                                om,
                                W3F[:, t, :, j, :],
                                BP[:, :, si, kh : kh + 28, kw + 1 : kw + 29]
                                .rearrange(
                                    "p c (y a) (x b) -> p c y a x b", a=2, b=2
                                )[:, :, :, 0, :, 0],
                                start=(t == 0),
                                stop=(t == 8),
                                perf_mode=DR,
                            )
                    # fused BN+RPReLU+sBN1: P1 = prelu(sA1*t + bA1, beta1)
                    P1 = p1pool.tile([128, NCOL], f32, tag=f"p1{j}")
                    nc.scalar.activation(
                        P1[:], ps1[:, :NCOL], Act.Prelu,
                        bias=cvec(2 + j), scale=cvec(0 + j), alpha=cvec(4 + j),
                    )
                    # y = E1*pool + P1 with exact-f32 E1 (bf16 E1 in the
                    # pool diag costs ~1% end-to-end via quant boundaries)
                    nc.vector.scalar_tensor_tensor(
                        y4[:, j, :], Q2p[j][:, :NCOL], cvec(30 + j), P1[:],
                        Alu.mult, Alu.add,
                    )
                    # stage-2 binarize; j=0 on scalar (+/-1), j=1 on DVE
                    # (+/-0.5, conv2 c1 weights doubled) to balance engines
                    if j == 0:
                        nc.scalar.activation(
                            S24[:, j, :], y4[:, j, :], Act.Sign,
                            bias=cvec(6 + j),
                        )
                    else:
                        nc.vector.tensor_scalar(
                            S24[:, j, :], y4[:, j, :], cvec(32), 0.5,
                            Alu.is_ge, Alu.subtract,
                        )

                # stage-2 shortcut quant: bf16 grid on y, clip
                R2 = qpool.tile([128, 2, NCOL], bf16, tag="r2")
                for j in range(2):
                    nc.vector.tensor_scalar(
                        R2[:, j, :], y4[:, j, :], 7.5, cvec(8 + j),
                        Alu.mult, Alu.add,
                    )
                RC2 = qpool.tile([128, 2, NCOL], bf16, tag="rc2")
                nc.vector.tensor_scalar(
                    RC2[:], R2[:], 207.0, 192.0, Alu.min, Alu.max
                )

                # stage 2: one fp8 DoubleRow matmul per output tile
                Z = zpool.tile([128, 4, NCOL], bf16, tag="z")
                for jj in range(4):
                    ps2 = pc2.tile([128, 512], f32, tag="ps2")
                    nc.tensor.matmul(
                        ps2[:, :NCOL],
                        W1F[:, :, jj, :],
                        S24[:],
                        start=True,
                        stop=True,
                        perf_mode=DR,
                    )
                    P2 = p2pool.tile([128, NCOL], bf16, tag="p2")
                    nc.scalar.activation(
                        P2[:], ps2[:, :NCOL], Act.Prelu,
                        bias=cvec(14 + jj), scale=cvec(10 + jj),
                        alpha=cvec(18 + jj),
                    )
                    U = p2pool.tile([128, NCOL], bf16, tag="u")
                    nc.vector.tensor_scalar(
                        U[:], RC2[:, jj % 2, :], cvec(22 + jj), cvec(26 + jj),
                        Alu.mult, Alu.add,
                    )
                    nc.vector.tensor_tensor(
                        Z[:, jj, :], U[:], P2[:], Alu.add
                    )

                nc.sync.dma_start(
                    out_d[:, :, NG * g : NG * g + NG, :].rearrange(
                        "jj p s x -> p jj (s x)"
                    ),
                    Z[:],
                )

    nc.compile()
    _PROGRAM_CACHE["nc"] = nc
    return nc


def _prep_consts(
    w3, w1,
    bn1_m, bn1_v, bn1_w, bn1_b,
    bn2_m, bn2_v, bn2_w, bn2_b,
    sbn1_m, sbn1_v, sbn1_w, sbn1_b,
    sbn2_m, sbn2_v, sbn2_w, sbn2_b,
    rp1_gamma, rp1_beta, rp1_zeta,
    rp2_gamma, rp2_beta, rp2_zeta,
):
    f = np.float32
    eps = f(1e-5)
    w3 = w3.astype(f)
    w1 = w1.astype(f)

    inv1 = bn1_w / np.sqrt(bn1_v + eps)
    shift1 = bn1_b - bn1_m * inv1
    alpha3 = np.mean(np.abs(w3), axis=(1, 2, 3))
    A1 = alpha3 * inv1
    base1 = shift1 - rp1_gamma
    sinv1 = sbn1_w / np.sqrt(sbn1_v + eps)
    sshift1 = sbn1_b - sbn1_m * sinv1
    sA1 = sinv1 * A1
    bA1 = sinv1 * base1
    E1 = sinv1 / f(30.0)
    D1 = sinv1 * rp1_zeta + sshift1 - f(798.0) * E1
    r2bias = f(199.5) + f(7.5) * D1

    inv2 = bn2_w / np.sqrt(bn2_v + eps)
    shift2 = bn2_b - bn2_m * inv2
    alpha1 = np.mean(np.abs(w1), axis=(1, 2, 3))
    A2 = alpha1 * inv2
    base2 = shift2 - rp2_gamma
    sinv2 = sbn2_w / np.sqrt(sbn2_v + eps)
    sshift2 = sbn2_b - sbn2_m * sinv2
    sA2 = sinv2 * A2
    bA2 = sinv2 * base2
    E2 = f(2.0 / 15.0) * sinv2
    D2 = sinv2 * rp2_zeta + sshift2 - f(199.5) * E2

    cv = np.zeros((128, 33), dtype=f)
    for j in range(2):
        sl = slice(j * 128, (j + 1) * 128)
        cv[:, 0 + j] = sA1[sl]
        cv[:, 2 + j] = bA1[sl]
        cv[:, 4 + j] = rp1_beta[sl]
        cv[:, 6 + j] = D1[sl]
        cv[:, 8 + j] = r2bias[sl]
        cv[:, 30 + j] = E1[sl]
    cv[:, 32] = -D1[128:]
    for jj in range(4):
        sl = slice(jj * 128, (jj + 1) * 128)
        cv[:, 10 + jj] = sA2[sl]
        cv[:, 14 + jj] = bA2[sl]
        cv[:, 18 + jj] = rp2_beta[sl]
        cv[:, 22 + jj] = E2[sl]
        cv[:, 26 + jj] = D2[sl]

    s3 = np.where(w3 >= 0, f(1.0), f(-1.0))
    # w3f[k, kh*3+kw, c, j, m] = s3[j*128+m, c*128+k, kh, kw]
    w3f = (
        s3.reshape(2, 128, 2, 128, 3, 3)
        .transpose(3, 4, 5, 2, 0, 1)
        .reshape(128, 9, 2, 2, 128)
        .astype(ml_dtypes.float8_e4m3)
    )
    s1 = np.where(w1 >= 0, f(1.0), f(-1.0))[:, :, 0, 0]
    # c=1 activations are +/-0.5 (DVE is_ge path) -> double those weights
    s1[:, 128:] *= f(2.0)
    # w1f[k, c, jj, m] = s1[jj*128+m, c*128+k]
    w1f = (
        s1.reshape(4, 128, 2, 128)
        .transpose(3, 2, 0, 1)
        .astype(ml_dtypes.float8_e4m3)
    )
    dg = np.eye(128, dtype=ml_dtypes.bfloat16)
    return w3f, w1f, dg, cv


def run(inputs, trace=False):
    from concourse import bass_utils

    nc = _build_program()
    x = np.asarray(inputs["x"], dtype=np.float32)
    w3f, w1f, dg, cv = _prep_consts(
        **{k: np.asarray(v, np.float32) for k, v in inputs.items() if k != "x"}
    )

    in_maps = []
    for core in range(N_CORES):
        xs = (
            x[core * B_PER_CORE : (core + 1) * B_PER_CORE]
            .reshape(B_PER_CORE, 2, 128, H * W)
            .copy()
        )
        in_maps.append({"xs": xs, "w3f": w3f, "w1f": w1f, "dg": dg, "cv": cv})

    res = bass_utils.run_bass_kernel_spmd(
        nc, in_maps, core_ids=list(range(N_CORES)), trace=trace
    )
    outs = [
        res.results[c]["out"]
        .astype(np.float32)
        .transpose(2, 0, 1, 3)
        .reshape(B_PER_CORE, COUT, HO, WO)
        for c in range(N_CORES)
    ]
    full = np.concatenate(outs, axis=0)
    return full, res


def kernel(**inputs):
    out, _ = run(inputs, trace=False)
    return out


# revision 32
# speedup vs baseline: 1.0620x; 1.0620x over previous
"""Trainium2 Bass kernel for the ReActNet-style binary conv building block.

Data-parallel across 8 NeuronCores (8 samples each). Key structure per
2-sample group:
  - scalar Sign act binarizes x -> fp8 +/-1 planes (zero-padded ring)
  - conv1 runs as 9 fp8 DoubleRow matmuls per half (K=256 per instr)
  - the 2x2 avgpool shortcut: bf16 quant grid (bf16(7.5x+199.5) rounds
    exactly), clipped, then summed by diag(E1) matmuls into PSUM
  - BN+RPReLU+shortcut-BN fold into a single per-channel Prelu
    activation (alpha = beta vector) per conv tile
  - stage 2: 1x1 conv as one fp8 DoubleRow matmul per 128-channel tile
  - final combine on DVE in bf16; output stored bf16, host casts to f32
"""

import sys

sys.path.insert(0, "/opt/trn_rl_repo")

import numpy as np
import ml_dtypes

B_PER_CORE = 8
N_CORES = 8
CIN = 256
COUT = 512
H = 28
W = 28
HO = 14
WO = 14
PIX = HO * WO  # 196
NG = 2  # samples per group
NGROUP = 4  # groups per core
NCOL = NG * PIX  # 392

# padded image layout rows 0..29, cols 0..31; interior at [1:29, 2:30]
PH, PW = 30, 32

_PROGRAM_CACHE = {}


def _build_program():
    if "nc" in _PROGRAM_CACHE:
        return _PROGRAM_CACHE["nc"]

    import concourse.bacc as bacc
    import concourse.tile as tile
    from concourse import mybir

    f32 = mybir.dt.float32
    bf16 = mybir.dt.bfloat16
    fp8 = mybir.dt.float8e4
    Alu = mybir.AluOpType
    Act = mybir.ActivationFunctionType
    DR = mybir.MatmulPerfMode.DoubleRow

    nc = bacc.Bacc(
        "TRN2",
        target_bir_lowering=False,
        debug=False,
        enable_asserts=False,
        num_devices=N_CORES,
    )

    xs_d = nc.dram_tensor("xs", [B_PER_CORE, 2, 128, H * W], f32, kind="ExternalInput")
    w3_d = nc.dram_tensor("w3f", [128, 9, 2, 2, 128], fp8, kind="ExternalInput")
    w1_d = nc.dram_tensor("w1f", [128, 2, 4, 128], fp8, kind="ExternalInput")
    dg_d = nc.dram_tensor("dg", [128, 128], bf16, kind="ExternalInput")
    cv_d = nc.dram_tensor("cv", [128, 33], f32, kind="ExternalInput")
    out_d = nc.dram_tensor(
        "out", [4, 128, B_PER_CORE, PIX], bf16, kind="ExternalOutput"
    )

    with tile.TileContext(nc) as tc:
        with (
            tc.tile_pool(name="consts", bufs=1) as cpool,
            tc.tile_pool(name="xin", bufs=2) as xpool,
            tc.tile_pool(name="rq", bufs=2) as rpool,
            tc.tile_pool(name="bpad", bufs=2) as bpool,
            tc.tile_pool(name="p1s", bufs=2) as p1pool,
            tc.tile_pool(name="ys", bufs=2) as ypool,
            tc.tile_pool(name="q2s", bufs=2) as qpool,
            tc.tile_pool(name="s2s", bufs=2) as spool,
            tc.tile_pool(name="p2s", bufs=2) as p2pool,
            tc.tile_pool(name="zs", bufs=2) as zpool,
            tc.tile_pool(name="pc1", bufs=2, space="PSUM") as pc1,
            tc.tile_pool(name="pq", bufs=2, space="PSUM") as pq,
            tc.tile_pool(name="pc2", bufs=2, space="PSUM") as pc2,
        ):
            W3F = cpool.tile([128, 9, 2, 2, 128], fp8)
            W1F = cpool.tile([128, 2, 4, 128], fp8)
            DG = cpool.tile([128, 128], bf16)
            CV = cpool.tile([128, 33], f32)
            nc.sync.dma_start(W3F[:], w3_d[:])
            nc.sync.dma_start(W1F[:], w1_d[:])
            nc.sync.dma_start(DG[:], dg_d[:])
            nc.sync.dma_start(CV[:], cv_d[:])

            def cvec(col):
                return CV[:, col : col + 1]

            # cv columns: 0,1 sA1 | 2,3 bA1 | 4,5 beta1 | 6,7 D1 | 8,9 r2bias
            # 10-13 sA2 | 14-17 bA2 | 18-21 beta2 | 22-25 E2 | 26-29 D2
            # 30,31 E1 | 32 -D1 (j=1, is_ge threshold)

            for g in range(NGROUP):
                X = xpool.tile([128, 2, NG, H * W], f32, tag="x")
                for si in range(NG):
                    nc.sync.dma_start(
                        X[:, :, si, :],
                        xs_d[NG * g + si].rearrange("c p hw -> p c hw"),
                    )
                Xv = X[:].rearrange("p c s (h w) -> p c s h w", h=H, w=W)

                BP = bpool.tile([128, 2, NG, PH, PW], fp8, tag="bpad")
                if g < 2:  # one memset per rotating buffer (bufs=2)
                    # ring cells the conv taps read; interior rewritten
                    # each group, ring stays zero across buffer reuse
                    nc.gpsimd.memset(BP[:, :, :, 0, :], 0.0)
                    nc.gpsimd.memset(BP[:, :, :, 1:29, 1], 0.0)
                # binarize: sign(x) in {-1,+1} -> fp8, zero-padded ring
                nc.scalar.activation(
                    BP[:, :, :, 1:29, 2:30], Xv, Act.Sign
                )

                # quant grid: bf16(7.5x+199.5) rounds to ints; clip [192,207]
                R = rpool.tile([128, 2, NG, H * W], bf16, tag="r")
                nc.vector.tensor_scalar(
                    R[:], X[:], 7.5, 199.5, Alu.mult, Alu.add
                )
                RC = rpool.tile([128, 2, NG, H, W], bf16, tag="rc")
                nc.vector.tensor_scalar(
                    RC[:], R[:], 207.0, 192.0, Alu.min, Alu.max
                )

                # 2x2 sum-pool via identity matmuls into PSUM (exact ints)
                Q2p = [
                    pq.tile([128, 512], f32, tag=f"pq{j}", name=f"q2p_{g}_{j}")
                    for j in range(2)
                ]
                for j in range(2):
                    om = Q2p[j][:, :NCOL].rearrange(
                        "p (s y x) -> p s y x", s=NG, y=HO
                    )
                    for pp in range(4):
                        ph, pw = pp >> 1, pp & 1
                        nc.tensor.matmul(
                            om,
                            DG[:],
                            RC[:, j, :, ph::2, pw::2],
                            start=(pp == 0),
                            stop=(pp == 3),
                        )

                # conv1: 9 fp8 DoubleRow matmuls per half (K=256 each)
                y4 = ypool.tile([128, 2, NCOL], f32, tag="y4")
                S24 = spool.tile([128, 2, NCOL], fp8, tag="s24")
                for j in range(2):
                    ps1 = pc1.tile([128, 512], f32, tag="ps1")
                    for si in range(NG):
                        om = ps1[:, si * PIX : (si + 1) * PIX].rearrange(
                            "p (y x) -> p y x", y=HO
                        )
                        for t in range(9):
                            kh, kw = t // 3, t % 3
                            nc.tensor.matmul(
                                om,
                                W3F[:, t, :, j, :],
                                BP[:, :, si, kh : kh + 28, kw + 1 : kw + 29]
                                .rearrange(
                                    "p c (y a) (x b) -> p c y a x b", a=2, b=2
                                )[:, :, :, 0, :, 0],
                                start=(t == 0),
                                stop=(t == 8),
                                perf_mode=DR,
                            )
                    # fused BN+RPReLU+sBN1: P1 = prelu(sA1*t + bA1, beta1)
                    P1 = p1pool.tile([128, NCOL], f32, tag=f"p1{j}")
                    nc.scalar.activation(
                        P1[:], ps1[:, :NCOL], Act.Prelu,
                        bias=cvec(2 + j), scale=cvec(0 + j), alpha=cvec(4 + j),
                    )
                    # y = E1*pool + P1 with exact-f32 E1 (bf16 E1 in the
                    # pool diag costs ~1% end-to-end via quant boundaries)
                    nc.vector.scalar_tensor_tensor(
                        y4[:, j, :], Q2p[j][:, :NCOL], cvec(30 + j), P1[:],
                        Alu.mult, Alu.add,
                    )
                    # stage-2 binarize: sign(y + D1) -> fp8
                    nc.scalar.activation(
                        S24[:, j, :], y4[:, j, :], Act.Sign, bias=cvec(6 + j)
                    )

                # stage-2 shortcut quant: bf16 grid on y, clip
                R2 = qpool.tile([128, 2, NCOL], bf16, tag="r2")
                for j in range(2):
                    nc.vector.tensor_scalar(
                        R2[:, j, :], y4[:, j, :], 7.5, cvec(8 + j),
                        Alu.mult, Alu.add,
                    )
                RC2 = qpool.tile([128, 2, NCOL], bf16, tag="rc2")
                nc.vector.tensor_scalar(
                    RC2[:], R2[:], 207.0, 192.0, Alu.min, Alu.max
                )

                # stage 2: one fp8 DoubleRow matmul per output tile
                Z = zpool.tile([128, 4, NCOL], bf16, tag="z")
                for jj in range(4):
                    ps2 = pc2.tile([128, 512], f32, tag="ps2")
                    nc.tensor.matmul(
                        ps2[:, :NCOL],
                        W1F[:, :, jj, :],
                        S24[:],
                        start=True,
                        stop=True,
                        perf_mode=DR,
                    )
                    P2 = p2pool.tile([128, NCOL], bf16, tag="p2")
                    nc.scalar.activation(
                        P2[:], ps2[:, :NCOL], Act.Prelu,
                        bias=cvec(14 + jj), scale=cvec(10 + jj),
                        alpha=cvec(18 + jj),
                    )
                    U = p2pool.tile([128, NCOL], bf16, tag="u")
                    nc.vector.tensor_scalar(
                        U[:], RC2[:, jj % 2, :], cvec(22 + jj), cvec(26 + jj),
                        Alu.mult, Alu.add,
                    )
                    nc.vector.tensor_tensor(
                        Z[:, jj, :], U[:], P2[:], Alu.add
                    )

                nc.sync.dma_start(
                    out_d[:, :, NG * g : NG * g + NG, :].rearrange(
                        "jj p s x -> p jj (s x)"
                    ),
                    Z[:],
                )

    nc.compile()
    _PROGRAM_CACHE["nc"] = nc
    return nc


def _prep_consts(
    w3, w1,
    bn1_m, bn1_v, bn1_w, bn1_b,
    bn2_m, bn2_v, bn2_w, bn2_b,
    sbn1_m, sbn1_v, sbn1_w, sbn1_b,
    sbn2_m, sbn2_v, sbn2_w, sbn2_b,
    rp1_gamma, rp1_beta, rp1_zeta,
    rp2_gamma, rp2_beta, rp2_zeta,
):
    f = np.float32
    eps = f(1e-5)
    w3 = w3.astype(f)
    w1 = w1.astype(f)

    inv1 = bn1_w / np.sqrt(bn1_v + eps)
    shift1 = bn1_b - bn1_m * inv1
    alpha3 = np.mean(np.abs(w3), axis=(1, 2, 3))
    A1 = alpha3 * inv1
    base1 = shift1 - rp1_gamma
    sinv1 = sbn1_w / np.sqrt(sbn1_v + eps)
    sshift1 = sbn1_b - sbn1_m * sinv1
    sA1 = sinv1 * A1
    bA1 = sinv1 * base1
    E1 = sinv1 / f(30.0)
    D1 = sinv1 * rp1_zeta + sshift1 - f(798.0) * E1
    r2bias = f(199.5) + f(7.5) * D1

    inv2 = bn2_w / np.sqrt(bn2_v + eps)
    shift2 = bn2_b - bn2_m * inv2
    alpha1 = np.mean(np.abs(w1), axis=(1, 2, 3))
    A2 = alpha1 * inv2
    base2 = shift2 - rp2_gamma
    sinv2 = sbn2_w / np.sqrt(sbn2_v + eps)
    sshift2 = sbn2_b - sbn2_m * sinv2
    sA2 = sinv2 * A2
    bA2 = sinv2 * base2
    E2 = f(2.0 / 15.0) * sinv2
    D2 = sinv2 * rp2_zeta + sshift2 - f(199.5) * E2

    cv = np.zeros((128, 33), dtype=f)
    for j in range(2):
        sl = slice(j * 128, (j + 1) * 128)
        cv[:, 0 + j] = sA1[sl]
        cv[:, 2 + j] = bA1[sl]
        cv[:, 4 + j] = rp1_beta[sl]
        cv[:, 6 + j] = D1[sl]
        cv[:, 8 + j] = r2bias[sl]
        cv[:, 30 + j] = E1[sl]
    cv[:, 32] = -D1[128:]
    for jj in range(4):
        sl = slice(jj * 128, (jj + 1) * 128)
        cv[:, 10 + jj] = sA2[sl]
        cv[:, 14 + jj] = bA2[sl]
        cv[:, 18 + jj] = rp2_beta[sl]
        cv[:, 22 + jj] = E2[sl]
        cv[:, 26 + jj] = D2[sl]

    s3 = np.where(w3 >= 0, f(1.0), f(-1.0))
    # w3f[k, kh*3+kw, c, j, m] = s3[j*128+m, c*128+k, kh, kw]
    w3f = (
        s3.reshape(2, 128, 2, 128, 3, 3)
        .transpose(3, 4, 5, 2, 0, 1)
        .reshape(128, 9, 2, 2, 128)
        .astype(ml_dtypes.float8_e4m3)
    )
    s1 = np.where(w1 >= 0, f(1.0), f(-1.0))[:, :, 0, 0]
    # w1f[k, c, jj, m] = s1[jj*128+m, c*128+k]
    w1f = (
        s1.reshape(4, 128, 2, 128)
        .transpose(3, 2, 0, 1)
        .astype(ml_dtypes.float8_e4m3)
    )
    dg = np.eye(128, dtype=ml_dtypes.bfloat16)
    return w3f, w1f, dg, cv


def run(inputs, trace=False):
    from concourse import bass_utils

    nc = _build_program()
    x = np.asarray(inputs["x"], dtype=np.float32)
    w3f, w1f, dg, cv = _prep_consts(
        **{k: np.asarray(v, np.float32) for k, v in inputs.items() if k != "x"}
    )

    in_maps = []
    for core in range(N_CORES):
        xs = (
            x[core * B_PER_CORE : (core + 1) * B_PER_CORE]
            .reshape(B_PER_CORE, 2, 128, H * W)
            .copy()
        )
        in_maps.append({"xs": xs, "w3f": w3f, "w1f": w1f, "dg": dg, "cv": cv})

    res = bass_utils.run_bass_kernel_spmd(
        nc, in_maps, core_ids=list(range(N_CORES)), trace=trace
    )
    outs = [
        res.results[c]["out"]
        .astype(np.float32)
        .transpose(2, 0, 1, 3)
        .reshape(B_PER_CORE, COUT, HO, WO)
        for c in range(N_CORES)
    ]
    full = np.concatenate(outs, axis=0)
    return full, res


def kernel(**inputs):
    out, _ = run(inputs, trace=False)
    return out


# revision 35
# speedup vs baseline: 1.1299x; 1.0639x over previous
"""Trainium2 Bass kernel for the ReActNet-style binary conv building block.

Data-parallel across 8 NeuronCores (8 samples each). Key structure per
2-sample group:
  - scalar Sign act binarizes x -> fp8 +/-1 planes (zero-padded ring)
  - conv1 runs as 9 fp8 DoubleRow matmuls per half (K=256 per instr)
  - the 2x2 avgpool shortcut: bf16 quant grid (bf16(7.5x+199.5) rounds
    exactly), clipped, then summed by diag(E1) matmuls into PSUM
  - BN+RPReLU+shortcut-BN fold into a single per-channel Prelu
    activation (alpha = beta vector) per conv tile
  - stage 2: 1x1 conv as one fp8 DoubleRow matmul per 128-channel tile
  - final combine on DVE in bf16; output stored bf16, host casts to f32
"""

import sys

sys.path.insert(0, "/opt/trn_rl_repo")

import numpy as np
import ml_dtypes

B_PER_CORE = 8
N_CORES = 8
CIN = 256
COUT = 512
H = 28
W = 28
HO = 14
WO = 14
PIX = HO * WO  # 196
NG = 2  # samples per group
NGROUP = 4  # groups per core
NCOL = NG * PIX  # 392

# padded image layout rows 0..29, cols 0..31; interior at [1:29, 2:30]
PH, PW = 30, 32

_PROGRAM_CACHE = {}


def _build_program():
    if "nc" in _PROGRAM_CACHE:
        return _PROGRAM_CACHE["nc"]

    import concourse.bacc as bacc
    import concourse.tile as tile
    from concourse import mybir

    f32 = mybir.dt.float32
    bf16 = mybir.dt.bfloat16
    fp8 = mybir.dt.float8e4
    Alu = mybir.AluOpType
    Act = mybir.ActivationFunctionType
    DR = mybir.MatmulPerfMode.DoubleRow

    nc = bacc.Bacc(
        "TRN2",
        target_bir_lowering=False,
        debug=False,
        enable_asserts=False,
        num_devices=N_CORES,
    )

    xs_d = nc.dram_tensor("xs", [B_PER_CORE, 2, 128, H * W], f32, kind="ExternalInput")
    w3_d = nc.dram_tensor("w3f", [128, 9, 2, 2, 128], fp8, kind="ExternalInput")
    w1_d = nc.dram_tensor("w1f", [128, 2, 4, 128], fp8, kind="ExternalInput")
    dg_d = nc.dram_tensor("dg", [128, 128], bf16, kind="ExternalInput")
    cv_d = nc.dram_tensor("cv", [128, 33], f32, kind="ExternalInput")
    out_d = nc.dram_tensor(
        "out", [4, 128, B_PER_CORE, PIX], bf16, kind="ExternalOutput"
    )

    with tile.TileContext(nc) as tc:
        with (
            tc.tile_pool(name="consts", bufs=1) as cpool,
            tc.tile_pool(name="xin", bufs=2) as xpool,
            tc.tile_pool(name="rq", bufs=2) as rpool,
            tc.tile_pool(name="bpad", bufs=2) as bpool,
            tc.tile_pool(name="p1s", bufs=2) as p1pool,
            tc.tile_pool(name="ys", bufs=2) as ypool,
            tc.tile_pool(name="q2s", bufs=2) as qpool,
            tc.tile_pool(name="s2s", bufs=2) as spool,
            tc.tile_pool(name="p2s", bufs=2) as p2pool,
            tc.tile_pool(name="zs", bufs=2) as zpool,
            tc.tile_pool(name="pc1", bufs=2, space="PSUM") as pc1,
            tc.tile_pool(name="pq", bufs=2, space="PSUM") as pq,
            tc.tile_pool(name="pc2", bufs=2, space="PSUM") as pc2,
        ):
            W3F = cpool.tile([128, 9, 2, 2, 128], fp8)
            W1F = cpool.tile([128, 2, 4, 128], fp8)
            DG = cpool.tile([128, 128], bf16)
            CV = cpool.tile([128, 33], f32)

            def cvec(col):
                return CV[:, col : col + 1]

            # cv columns: 0,1 sA1 | 2,3 bA1 | 4,5 beta1 | 6,7 D1 | 8,9 r2bias
            # 10-13 sA2 | 14-17 bA2 | 18-21 beta2 | 22-25 E2 | 26-29 D2
            # 30,31 E1 | 32 -D1 (j=1, is_ge threshold)

            for g in range(NGROUP):
                X = xpool.tile([128, 2, NG, H * W], f32, tag="x")
                for si in range(NG):
                    nc.sync.dma_start(
                        X[:, :, si, :],
                        xs_d[NG * g + si].rearrange("c p hw -> p c hw"),
                    )
                if g == 0:
                    # weights queue AFTER g0's input: the binarize gates
                    # everything, while matmuls start ~3us later
                    nc.sync.dma_start(DG[:], dg_d[:])
                    nc.sync.dma_start(CV[:], cv_d[:])
                    nc.sync.dma_start(W3F[:], w3_d[:])
                    nc.sync.dma_start(W1F[:], w1_d[:])
                Xv = X[:].rearrange("p c s (h w) -> p c s h w", h=H, w=W)

                BP = bpool.tile([128, 2, NG, PH, PW], fp8, tag="bpad")
                if g < 2:  # one memset per rotating buffer (bufs=2)
                    # ring cells the conv taps read; interior rewritten
                    # each group, ring stays zero across buffer reuse
                    nc.gpsimd.memset(BP[:, :, :, 0, :], 0.0)
                    nc.gpsimd.memset(BP[:, :, :, 1:29, 1], 0.0)
                # binarize: sign(x) in {-1,+1} -> fp8, zero-padded ring
                nc.scalar.activation(
                    BP[:, :, :, 1:29, 2:30], Xv, Act.Sign
                )

                # quant grid: bf16(7.5x+199.5) rounds to ints; clip [192,207]
                R = rpool.tile([128, 2, NG, H * W], bf16, tag="r")
                nc.vector.tensor_scalar(
                    R[:], X[:], 7.5, 199.5, Alu.mult, Alu.add
                )
                RC = rpool.tile([128, 2, NG, H, W], bf16, tag="rc")
                nc.vector.tensor_scalar(
                    RC[:], R[:], 207.0, 192.0, Alu.min, Alu.max
                )

                # 2x2 sum-pool via identity matmuls into PSUM (exact ints)
                Q2p = [
                    pq.tile([128, 512], f32, tag=f"pq{j}", name=f"q2p_{g}_{j}")
                    for j in range(2)
                ]
                for j in range(2):
                    om = Q2p[j][:, :NCOL].rearrange(
                        "p (s y x) -> p s y x", s=NG, y=HO
                    )
                    for pp in range(4):
                        ph, pw = pp >> 1, pp & 1
                        nc.tensor.matmul(
                            om,
                            DG[:],
                            RC[:, j, :, ph::2, pw::2],
                            start=(pp == 0),
                            stop=(pp == 3),
                        )

                # conv1: 9 fp8 DoubleRow matmuls per half (K=256 each)
                y4 = ypool.tile([128, 2, NCOL], f32, tag="y4")
                S24 = spool.tile([128, 2, NCOL], fp8, tag="s24")
                for j in range(2):
                    ps1 = pc1.tile([128, 512], f32, tag="ps1")
                    for si in range(NG):
                        om = ps1[:, si * PIX : (si + 1) * PIX].rearrange(
                            "p (y x) -> p y x", y=HO
                        )
                        for t in range(9):
                            kh, kw = t // 3, t % 3
                            nc.tensor.matmul(
                                om,
                                W3F[:, t, :, j, :],
                                BP[:, :, si, kh : kh + 28, kw + 1 : kw + 29]
                                .rearrange(
                                    "p c (y a) (x b) -> p c y a x b", a=2, b=2
                                )[:, :, :, 0, :, 0],
                                start=(t == 0),
                                stop=(t == 8),
                                perf_mode=DR,
                            )
                    # fused BN+RPReLU+sBN1: P1 = prelu(sA1*t + bA1, beta1)
                    P1 = p1pool.tile([128, NCOL], f32, tag=f"p1{j}")
                    nc.scalar.activation(
                        P1[:], ps1[:, :NCOL], Act.Prelu,
                        bias=cvec(2 + j), scale=cvec(0 + j), alpha=cvec(4 + j),
                    )
                    # y = E1*pool + P1 with exact-f32 E1 (bf16 E1 in the
                    # pool diag costs ~1% end-to-end via quant boundaries)
                    nc.vector.scalar_tensor_tensor(
                        y4[:, j, :], Q2p[j][:, :NCOL], cvec(30 + j), P1[:],
                        Alu.mult, Alu.add,
                    )
                    # stage-2 binarize: sign(y + D1) -> fp8
                    nc.scalar.activation(
                        S24[:, j, :], y4[:, j, :], Act.Sign, bias=cvec(6 + j)
                    )

                # stage-2 shortcut quant: bf16 grid on y, clip
                R2 = qpool.tile([128, 2, NCOL], bf16, tag="r2")
                for j in range(2):
                    nc.vector.tensor_scalar(
                        R2[:, j, :], y4[:, j, :], 7.5, cvec(8 + j),
                        Alu.mult, Alu.add,
                    )
                RC2 = qpool.tile([128, 2, NCOL], bf16, tag="rc2")
                nc.vector.tensor_scalar(
                    RC2[:], R2[:], 207.0, 192.0, Alu.min, Alu.max
                )

                # stage 2: one fp8 DoubleRow matmul per output tile
                Z = zpool.tile([128, 4, NCOL], bf16, tag="z")
                for jj in range(4):
                    ps2 = pc2.tile([128, 512], f32, tag="ps2")
                    nc.tensor.matmul(
                        ps2[:, :NCOL],
                        W1F[:, :, jj, :],
                        S24[:],
                        start=True,
                        stop=True,
                        perf_mode=DR,
                    )
                    P2 = p2pool.tile([128, NCOL], bf16, tag="p2")
                    nc.scalar.activation(
                        P2[:], ps2[:, :NCOL], Act.Prelu,
                        bias=cvec(14 + jj), scale=cvec(10 + jj),
                        alpha=cvec(18 + jj),
                    )
                    U = p2pool.tile([128, NCOL], bf16, tag="u")
                    nc.vector.tensor_scalar(
                        U[:], RC2[:, jj % 2, :], cvec(22 + jj), cvec(26 + jj),
                        Alu.mult, Alu.add,
                    )
                    nc.vector.tensor_tensor(
                        Z[:, jj, :], U[:], P2[:], Alu.add
                    )
                    nc.sync.dma_start(
                        out_d[jj, :, NG * g : NG * g + NG, :].rearrange(
                            "p s x -> p (s x)"
                        ),
                        Z[:, jj, :],
                    )

    nc.compile()
    _PROGRAM_CACHE["nc"] = nc
    return nc


def _prep_consts(
    w3, w1,
    bn1_m, bn1_v, bn1_w, bn1_b,
    bn2_m, bn2_v, bn2_w, bn2_b,
    sbn1_m, sbn1_v, sbn1_w, sbn1_b,
    sbn2_m, sbn2_v, sbn2_w, sbn2_b,
    rp1_gamma, rp1_beta, rp1_zeta,
    rp2_gamma, rp2_beta, rp2_zeta,
):
    f = np.float32
    eps = f(1e-5)
    w3 = w3.astype(f)
    w1 = w1.astype(f)

    inv1 = bn1_w / np.sqrt(bn1_v + eps)
    shift1 = bn1_b - bn1_m * inv1
    alpha3 = np.mean(np.abs(w3), axis=(1, 2, 3))
    A1 = alpha3 * inv1
    base1 = shift1 - rp1_gamma
    sinv1 = sbn1_w / np.sqrt(sbn1_v + eps)
    sshift1 = sbn1_b - sbn1_m * sinv1
    sA1 = sinv1 * A1
    bA1 = sinv1 * base1
    E1 = sinv1 / f(30.0)
    D1 = sinv1 * rp1_zeta + sshift1 - f(798.0) * E1
    r2bias = f(199.5) + f(7.5) * D1

    inv2 = bn2_w / np.sqrt(bn2_v + eps)
    shift2 = bn2_b - bn2_m * inv2
    alpha1 = np.mean(np.abs(w1), axis=(1, 2, 3))
    A2 = alpha1 * inv2
    base2 = shift2 - rp2_gamma
    sinv2 = sbn2_w / np.sqrt(sbn2_v + eps)
    sshift2 = sbn2_b - sbn2_m * sinv2
    sA2 = sinv2 * A2
    bA2 = sinv2 * base2
    E2 = f(2.0 / 15.0) * sinv2
    D2 = sinv2 * rp2_zeta + sshift2 - f(199.5) * E2

    cv = np.zeros((128, 33), dtype=f)
    for j in range(2):
        sl = slice(j * 128, (j + 1) * 128)
        cv[:, 0 + j] = sA1[sl]
        cv[:, 2 + j] = bA1[sl]
        cv[:, 4 + j] = rp1_beta[sl]
        cv[:, 6 + j] = D1[sl]
        cv[:, 8 + j] = r2bias[sl]
        cv[:, 30 + j] = E1[sl]
    cv[:, 32] = -D1[128:]
    for jj in range(4):
        sl = slice(jj * 128, (jj + 1) * 128)
        cv[:, 10 + jj] = sA2[sl]
        cv[:, 14 + jj] = bA2[sl]
        cv[:, 18 + jj] = rp2_beta[sl]
        cv[:, 22 + jj] = E2[sl]
        cv[:, 26 + jj] = D2[sl]

    s3 = np.where(w3 >= 0, f(1.0), f(-1.0))
    # w3f[k, kh*3+kw, c, j, m] = s3[j*128+m, c*128+k, kh, kw]
    w3f = (
        s3.reshape(2, 128, 2, 128, 3, 3)
        .transpose(3, 4, 5, 2, 0, 1)
        .reshape(128, 9, 2, 2, 128)
        .astype(ml_dtypes.float8_e4m3)
    )
    s1 = np.where(w1 >= 0, f(1.0), f(-1.0))[:, :, 0, 0]
    # w1f[k, c, jj, m] = s1[jj*128+m, c*128+k]
    w1f = (
        s1.reshape(4, 128, 2, 128)
        .transpose(3, 2, 0, 1)
        .astype(ml_dtypes.float8_e4m3)
    )
    dg = np.eye(128, dtype=ml_dtypes.bfloat16)
    return w3f, w1f, dg, cv


def run(inputs, trace=False):
    from concourse import bass_utils

    nc = _build_program()
    x = np.asarray(inputs["x"], dtype=np.float32)
    w3f, w1f, dg, cv = _prep_consts(
        **{k: np.asarray(v, np.float32) for k, v in inputs.items() if k != "x"}
    )

    in_maps = []
    for core in range(N_CORES):
        xs = (
            x[core * B_PER_CORE : (core + 1) * B_PER_CORE]
            .reshape(B_PER_CORE, 2, 128, H * W)
            .copy()
        )
        in_maps.append({"xs": xs, "w3f": w3f, "w1f": w1f, "dg": dg, "cv": cv})

    res = bass_utils.run_bass_kernel_spmd(
        nc, in_maps, core_ids=list(range(N_CORES)), trace=trace
    )
    outs = [
        res.results[c]["out"]
        .astype(np.float32)
        .transpose(2, 0, 1, 3)
        .reshape(B_PER_CORE, COUT, HO, WO)
        for c in range(N_CORES)
    ]
    full = np.concatenate(outs, axis=0)
    return full, res


def kernel(**inputs):
    out, _ = run(inputs, trace=False)
    return out
